# revision 9
# baseline (speedup 1.0000x reference)
"""DiffusionConv (Chebyshev graph diffusion conv) Trainium2 kernel, 8 NeuronCores.

Math (faithful to the reference's raw reshapes):
  x0 = x.reshape(n, c*b)                         # (10000, 4096)
  for each support s: x1_s = A_s @ x0 ; x2_s = 2*A_s@x1_s - x0
  xcat[b, n_, c*M + m] = xs[m][n_, c*64 + b]     # xs = [x0, x1_0, x2_0, x1_1, x2_1]
  out = xcat @ Theta + bias                      # (64, 10000, 64)

Sharding: 1D row-parallel spmm over the NODE dimension — core k owns dest rows
[k*1280, (k+1)*1280) of the (padded to 10240) node axis, with the full 4096
feature columns.  x0 is replicated, so round 1 (x1_s = A_s @ x0) needs no
communication; the x1_s row shards are AllGathered (fp8, 5.2MB/core) before
round 2 (z_s = A_s @ x1_s).  Each spmm is a dma_gather of full source rows
(fp8, 4KB/row — 8x fewer descriptors than batch sharding) + one-hot PE matmul
segment-sum into a [128, 4096] PSUM per dest block.  Scales (32x on S, powers
folded into Theta) keep fp8 ranges healthy; Chebyshev (2*A@x1 - x0) and the
(c, m) interleave fold into host-side Theta prep.  The final projection is
data-parallel over the core's node rows: K-tiles are loaded via DMA-transpose
from the bf16 y tensors (no PE transposes anywhere).
"""

import os
from contextlib import ExitStack

import numpy as np
import ml_dtypes

import concourse.bass as bass
import concourse.bacc as bacc
import concourse.tile as tile
import concourse.mybir as mybir

# ---- problem constants (hardcoded per contest rules) ----
N_NODES = 10000
N_EDGES = 320000
N_SUPPORTS = 2
C_IN = 64
C_OUT = 64
BATCH = 64
NCORES = 8
NBP = BATCH // 2                # 32 batch-pairs (all batches on every core)
FULLC = C_IN * BATCH            # 4096 feature columns, col = b*64 + c
NBLK_G = 80                     # global 128-row dest blocks (nodes padded)
NPAD_G = NBLK_G * 128           # 10240
BLKC = NBLK_G // NCORES         # 10 dest blocks per core
ROWSC = BLKC * 128              # 1280 dest rows per core

GATHER_B = 512                  # slots per dma_gather call
TPC = GATHER_B // 128           # S tiles per gather call
NCHUNK = FULLC // 512           # 8 psum column chunks per dest block

SSCALE = 32.0                   # S stored as 32*S in fp8 (psum = 32*x1)
NMAT = 5                        # [x0, x1_0, z_0, x1_1, z_1]

bf16 = mybir.dt.bfloat16
f32 = mybir.dt.float32
fp8 = mybir.dt.float8e4
i16 = mybir.dt.int16

E4NP = mybir.dt.np(mybir.dt.float8e4)

LAST_RESULT = {}


# --------------------------------------------------------------------------
# host-side edge preprocessing
# --------------------------------------------------------------------------
def _prep_edges(rows, cols, vals):
    """Per dest 128-block: dedup source cols into gather SLOTS (sorted
    ascending for HBM locality).  Every block is padded to the same tile
    count T (uniform schedule across cores).  Returns per-block
    (idx [T*128], S [T*128, 128]) lists plus T."""
    rows = np.asarray(rows, np.int64)
    cols = np.asarray(cols, np.int64)
    vals = np.asarray(vals, np.float32)
    blk = rows >> 7
    order = np.argsort(blk, kind="stable")
    r_s, c_s, v_s = rows[order], cols[order], vals[order]
    blk = r_s >> 7
    counts = np.bincount(blk, minlength=NBLK_G)
    per_blk = []
    start = 0
    for I in range(NBLK_G):
        cnt = int(counts[I])
        sl = slice(start, start + cnt)
        start += cnt
        uc, inv = np.unique(c_s[sl], return_inverse=True)
        per_blk.append((uc, inv, r_s[sl] - I * 128, v_s[sl]))
    T = max((len(uc) + 127) // 128 for uc, _, _, _ in per_blk)
    # tiles per core-spmm (BLKC*T) must divide into whole gather calls
    while (BLKC * T) % TPC:
        T += 1
    out = []
    for uc, inv, rloc, v in per_blk:
        npad = T * 128
        idx = np.zeros(npad, np.int64)
        idx[:len(uc)] = uc
        S_blk = np.zeros((npad, 128), np.float32)
        np.add.at(S_blk, (inv, rloc), v)
        out.append((idx, S_blk))
    return out, T


def _pack_core(per_blk, k, T):
    """Pack core k's BLKC blocks into uniform iw/s8 call tensors."""
    idx = np.concatenate([per_blk[k * BLKC + i][0] for i in range(BLKC)])
    s_all = np.concatenate([per_blk[k * BLKC + i][1] for i in range(BLKC)], 0)
    ncalls = (BLKC * T) // TPC
    iw = idx.reshape(ncalls, GATHER_B // 16, 16).transpose(0, 2, 1)
    iw = np.tile(iw, (1, 8, 1)).astype(np.int16)
    s8 = (s_all.reshape(ncalls, TPC, 128, 128).transpose(0, 2, 1, 3)
          .reshape(ncalls, 128, TPC * 128) * SSCALE).astype(E4NP)
    return np.ascontiguousarray(iw), np.ascontiguousarray(s8)


def _prep_theta(Theta, bias):
    """Fold Chebyshev affine, (c,m) interleave and fp8 scales into the 5
    effective K-tiles, duplicated across both partition halves (h=0/1)."""
    Theta = np.asarray(Theta, np.float64).reshape(C_IN, 5, C_OUT)  # [c, m, co]
    m = [Theta[:, i, :] for i in range(5)]
    S2 = SSCALE * SSCALE
    th = np.stack([m[0] - m[2] - m[4], m[1] / SSCALE, 2.0 * m[2] / S2,
                   m[3] / SSCALE, 2.0 * m[4] / S2], 0)   # [5, 64, 64]
    th_half = th.transpose(1, 0, 2).reshape(C_IN, 5 * C_OUT)  # [c, (m co)]
    th_full = np.concatenate([th_half, th_half], 0)            # [128, 320]
    b = np.asarray(bias, np.float64).reshape(1, C_OUT)
    c = ml_dtypes.bfloat16
    return th_full.astype(c), b.astype(c)


# --------------------------------------------------------------------------
# device program (identical on all 8 cores; inputs differ per core)
# --------------------------------------------------------------------------
def _emit_spmm(nc, pools, src_ap, y8_ap, ybf_ap, iw_ap, s8_ap, T):
    """One spmm over this core's BLKC dest blocks, full 4096 columns.
    psum[r, :] += sum_slots 32*S[slot, r] * src[idx_slot, :].
    Epilogue: fp8 y8 rows (if y8_ap) and bf16 ybf rows."""
    (iw_pool, g_pool, s_pool, y_pool, yb_pool, ps_y, const) = pools
    nreg = const["nreg"]
    ntiles = BLKC * T

    cur_psum = None
    G = None
    sc = None
    for t in range(ntiles):
        c, g = divmod(t, TPC)
        I, tt = divmod(t, T)
        if g == 0:
            it = iw_pool.tile([128, GATHER_B // 16], i16, tag="iw")
            nc.sync.dma_start(it[:], iw_ap[c])
            G = g_pool.tile([128, TPC, FULLC], fp8, tag="G")
            nc.gpsimd.dma_gather(G[:], src_ap, it[:], GATHER_B, nreg,
                                 FULLC, queue_num=c % 4)
            sc = s_pool.tile([128, TPC * 128], fp8, tag="S")
            nc.scalar.dma_start(sc[:], s8_ap[c])
        if tt == 0:
            cur_psum = ps_y.tile([128, FULLC], f32, tag="psy")
        first, last = tt == 0, tt == T - 1
        for ch in range(NCHUNK):
            cs = slice(ch * 512, (ch + 1) * 512)
            nc.tensor.matmul(cur_psum[:, cs], sc[:, g * 128:(g + 1) * 128],
                             G[:, g, cs], start=first, stop=last)
        if last:
            rows = slice(I * 128, (I + 1) * 128)
            if y8_ap is not None:
                y8 = y_pool.tile([128, FULLC], fp8, tag="y8")
                for ch in range(NCHUNK):
                    cs = slice(ch * 512, (ch + 1) * 512)
                    nc.vector.tensor_copy(y8[:, cs], cur_psum[:, cs])
                nc.sync.dma_start(y8_ap[rows, :], y8[:])
            ybf = yb_pool.tile([128, FULLC], bf16, tag="ybf")
            for ch in range(NCHUNK):
                cs = slice(ch * 512, (ch + 1) * 512)
                nc.vector.tensor_copy(ybf[:, cs], cur_psum[:, cs])
            nc.scalar.dma_start(ybf_ap[rows, :], ybf[:])


def _build_program(T):
    nc = bacc.Bacc("TRN2", target_bir_lowering=False, debug=False,
                   num_swdge_queues=4, dynamic_dma_scratch_size=32768,
                   num_devices=NCORES)
    ncalls = (BLKC * T) // TPC

    # inputs
    x0g = nc.dram_tensor("x0g", [NPAD_G, FULLC], fp8,
                         kind="ExternalInput").ap()
    x0K = nc.dram_tensor("x0K", [NBP, BLKC, 128, 128], bf16,
                         kind="ExternalInput").ap()
    iw, s8 = [], []
    for s in range(N_SUPPORTS):
        iw.append(nc.dram_tensor(f"iw{s}", [ncalls, 128, GATHER_B // 16], i16,
                                 kind="ExternalInput").ap())
        s8.append(nc.dram_tensor(f"s8{s}", [ncalls, 128, TPC * 128], fp8,
                                 kind="ExternalInput").ap())
    thd = nc.dram_tensor("thd", [128, NMAT * C_OUT], bf16,
                         kind="ExternalInput").ap()
    bias_d = nc.dram_tensor("biasd", [1, C_OUT], bf16,
                            kind="ExternalInput").ap()

    # internal DRAM
    y8loc = [nc.dram_tensor(f"y8loc{s}", [ROWSC, FULLC], fp8).ap()
             for s in range(N_SUPPORTS)]
    y8full = [nc.dram_tensor(f"y8full{s}", [NPAD_G, FULLC], fp8,
                             addr_space="Shared").ap()
              for s in range(N_SUPPORTS)]
    # projection K-tile sources (bf16, local rows): m = 1..4
    ybf = [nc.dram_tensor(f"ybf{m}", [ROWSC, FULLC], bf16).ap()
           for m in range(1, NMAT)]

    # output
    out_d = nc.dram_tensor("out", [NBP, BLKC, 128, 128], f32,
                           kind="ExternalOutput").ap()

    part = os.environ.get("KPART", "full")
    groups = [list(range(NCORES))]

    with tile.TileContext(nc) as tc, ExitStack() as ctx:
        const_p = ctx.enter_context(tc.tile_pool(name="const", bufs=1))
        th_sb = const_p.tile([128, NMAT * C_OUT], bf16)
        nc.sync.dma_start(th_sb[:], thd[:])
        bias_sb = const_p.tile([1, C_OUT], bf16)
        nc.sync.dma_start(bias_sb[:], bias_d[:])
        ones_sb = const_p.tile([1, 128], bf16)
        nc.vector.memset(ones_sb[:], 1.0)
        nreg = nc.gpsimd.to_reg(GATHER_B)
        const = {"nreg": nreg}

        # ---- spmm phase (own PSUM scope: releases all 8 banks afterwards) --
        if part != "proj":
            with tc.tile_pool(name="iw", bufs=4) as iw_pool, \
                 tc.tile_pool(name="g", bufs=4) as g_pool, \
                 tc.tile_pool(name="s", bufs=6) as s_pool, \
                 tc.tile_pool(name="y", bufs=2) as y_pool, \
                 tc.tile_pool(name="yb", bufs=2) as yb_pool, \
                 tc.tile_pool(name="psy", bufs=1, space="PSUM") as ps_y:
                pools = (iw_pool, g_pool, s_pool, y_pool, yb_pool, ps_y, const)
                # round 1: x1_s = A_s @ x0 -> y8loc_s (fp8) + ybf (bf16)
                _emit_spmm(nc, pools, x0g, y8loc[0], ybf[0], iw[0], s8[0], T)
                nc.gpsimd.collective_compute(
                    "AllGather", mybir.AluOpType.bypass,
                    replica_groups=groups, ins=[y8loc[0]], outs=[y8full[0]])
                _emit_spmm(nc, pools, x0g, y8loc[1], ybf[2], iw[1], s8[1], T)
                nc.gpsimd.collective_compute(
                    "AllGather", mybir.AluOpType.bypass,
                    replica_groups=groups, ins=[y8loc[1]], outs=[y8full[1]])
                # round 2: z_s = A_s @ x1_s -> ybf only
                if part != "spmm2":
                    _emit_spmm(nc, pools, y8full[0], None, ybf[1],
                               iw[0], s8[0], T)
                    _emit_spmm(nc, pools, y8full[1], None, ybf[3],
                               iw[1], s8[1], T)

        # ---- projection ---------------------------------------------------
        with tc.tile_pool(name="xc", bufs=6) as xc_pool, \
             tc.tile_pool(name="o", bufs=3) as o_pool, \
             tc.tile_pool(name="pso", bufs=2, space="PSUM") as ps_o:
            for bp in range(NBP):
                for I in range(BLKC):
                    rows = slice(I * 128, (I + 1) * 128)
                    cols = slice(bp * 128, (bp + 1) * 128)
                    xcx = xc_pool.tile([128, 128], bf16, tag="xcx")
                    nc.sync.dma_start(xcx[:], x0K[bp, I])
                    xcm = xc_pool.tile([128, 4, 128], bf16, tag="xcm")
                    for m in range(4):
                        eng = nc.sync if m % 2 else nc.scalar
                        eng.dma_start(xcm[:, m, :], ybf[m][rows, cols],
                                      transpose=True)
                    po = ps_o.tile([128, 128], f32, tag="pso")
                    for h in range(2):
                        hs = slice(h * 64, (h + 1) * 64)
                        pslice = po[:, h * C_OUT:(h + 1) * C_OUT]
                        nc.tensor.matmul(pslice, xcx[hs, :],
                                         th_sb[hs, 0:C_OUT],
                                         start=True, stop=False)
                        for m in range(4):
                            nc.tensor.matmul(
                                pslice, xcm[hs, m, :],
                                th_sb[hs, (m + 1) * C_OUT:(m + 2) * C_OUT],
                                start=False, stop=False)
                        nc.tensor.matmul(pslice, ones_sb[:], bias_sb[:],
                                         start=False, stop=True)
                    ob = o_pool.tile([128, 128], f32, tag="ob")
                    nc.vector.tensor_copy(ob[:], po[:])
                    nc.sync.dma_start(out_d[bp, I], ob[:])
    nc.compile()
    return nc


# --------------------------------------------------------------------------
# public entry point
# --------------------------------------------------------------------------
def kernel(x, edge_vals, Theta, bias, edge_rows, edge_cols):
    x = np.ascontiguousarray(np.asarray(x, np.float32))
    edge_vals = np.asarray(edge_vals, np.float32)
    edge_rows = np.asarray(edge_rows, np.int32)
    edge_cols = np.asarray(edge_cols, np.int32)

    # ---- host prep ----
    # raw-reshape x0 matrix; reorder cols from (c,b) to (b,c)
    x0m = x.reshape(N_NODES, FULLC)
    x0bc = np.ascontiguousarray(
        x0m.reshape(N_NODES, C_IN, BATCH).transpose(0, 2, 1)
    ).reshape(N_NODES, FULLC)
    x0p = np.zeros((NPAD_G, FULLC), np.float32)
    x0p[:N_NODES] = x0bc
    x0g_np = x0p.astype(E4NP)

    th_np, bias_np = _prep_theta(Theta, bias)

    per_blk_s, T_s = [], []
    for s in range(N_SUPPORTS):
        pb, T = _prep_edges(edge_rows[s], edge_cols[s], edge_vals[s])
        per_blk_s.append(pb)
        T_s.append(T)
    T = max(T_s)
    while (BLKC * T) % TPC:
        T += 1
    # re-pad supports to common T
    for s in range(N_SUPPORTS):
        if T_s[s] != T:
            pb = []
            for idx, S_blk in per_blk_s[s]:
                idx2 = np.zeros(T * 128, np.int64)
                idx2[:len(idx)] = idx
                S2 = np.zeros((T * 128, 128), np.float32)
                S2[:len(S_blk)] = S_blk
                pb.append((idx2, S2))
            per_blk_s[s] = pb

    nc = _build_program(T)

    in_maps = []
    for k in range(NCORES):
        # x0K[bp, I, h*64+c, j] = x0bc[k*1280 + I*128 + j, bp*128 + h*64+c]
        xr = x0p[k * ROWSC:(k + 1) * ROWSC]
        x0K_np = np.ascontiguousarray(
            xr.reshape(BLKC, 128, NBP, 128).transpose(2, 0, 3, 1)
        ).astype(ml_dtypes.bfloat16)
        im = {"x0g": x0g_np, "x0K": np.asarray(x0K_np),
              "thd": np.asarray(th_np), "biasd": np.asarray(bias_np)}
        for s in range(N_SUPPORTS):
            iw, s8 = _pack_core(per_blk_s[s], k, T)
            im[f"iw{s}"] = iw
            im[f"s8{s}"] = s8
        in_maps.append(im)

    results = _run_pjrt(nc, in_maps)

    # ---- host assembly ----
    out = np.empty((BATCH, N_NODES, C_OUT), np.float32)
    for k in range(NCORES):
        ok = results[k]["out"]  # [NBP, BLKC, 128, 128]
        r0 = k * ROWSC
        nrow = min(ROWSC, N_NODES - r0)
        if nrow <= 0:
            continue
        # [NBP, rows, 2, C_OUT] -> [2bp+h, node, co]
        okr = (ok.reshape(NBP, ROWSC, 2, C_OUT)[:, :nrow]
               .transpose(0, 2, 1, 3).reshape(BATCH, nrow, C_OUT))
        out[:, r0:r0 + nrow, :] = okr
    return out


# --------------------------------------------------------------------------
# PJRT execution (axon) — vendored from bass2jax.run_bass_via_pjrt, but
# without output-buffer donation so the compiled executable can be
# re-dispatched for timing (our kernel fully writes its output tensor).
# --------------------------------------------------------------------------
def _run_pjrt(nc, in_maps):
    import jax
    from jax.sharding import Mesh, PartitionSpec, NamedSharding
    from jax.experimental.shard_map import shard_map
    from concourse import bass2jax
    from concourse import mybir as mb

    bass2jax.install_neuronx_cc_hook()
    n_cores = len(in_maps)
    partition_name = (nc.partition_id_tensor.name
                      if nc.partition_id_tensor else None)

    in_names, out_names, out_avals, zero_outs = [], [], [], []
    for alloc in nc.m.functions[0].allocations:
        if not isinstance(alloc, mb.MemoryLocationSet):
            continue
        name = alloc.memorylocations[0].name
        if alloc.kind == "ExternalInput":
            if name != partition_name:
                in_names.append(name)
        elif alloc.kind == "ExternalOutput":
            out_names.append(name)
            shape = tuple(alloc.tensor_shape)
            dtype = mb.dt.np(alloc.dtype)
            out_avals.append(jax.core.ShapedArray(shape, dtype))
            zero_outs.append(np.zeros(shape, dtype))
    n_params = len(in_names)
    in_names.extend(out_names)
    if partition_name is not None:
        in_names.append(partition_name)

    def _body(*args):
        operands = list(args)
        if partition_name is not None:
            operands.append(bass2jax.partition_id_tensor())
        outs = bass2jax._bass_exec_p.bind(
            *operands,
            out_avals=tuple(out_avals),
            in_names=tuple(in_names),
            out_names=tuple(out_names),
            lowering_input_output_aliases=(),
            sim_require_finite=True,
            sim_require_nnan=True,
            nc=nc,
        )
        return tuple(outs)

    devices = jax.devices()[:n_cores]
    mesh = Mesh(np.asarray(devices), ("core",))
    in_specs = (PartitionSpec("core"),) * (n_params + len(out_names))
    out_specs = (PartitionSpec("core"),) * len(out_names)
    sharded = jax.jit(
        shard_map(_body, mesh=mesh, in_specs=in_specs, out_specs=out_specs,
                  check_rep=False),
        keep_unused=True,
    )
    per_core = [[np.asarray(m[name]) for name in in_names[:n_params]]
                for m in in_maps]
    sh = NamedSharding(mesh, PartitionSpec("core"))
    concat_in = [
        jax.device_put(
            np.concatenate([per_core[c][i] for c in range(n_cores)], axis=0),
            sh)
        for i in range(n_params)
    ]
    concat_zeros = [
        jax.device_put(np.zeros((n_cores * z.shape[0], *z.shape[1:]), z.dtype),
                       sh)
        for z in zero_outs
    ]
    out_arrs = sharded(*concat_in, *concat_zeros)
    jax.block_until_ready(out_arrs)
    LAST_RESULT["runner"] = (sharded, concat_in, concat_zeros)
    return [
        {name: np.asarray(out_arrs[i]).reshape(n_cores, *out_avals[i].shape)[c]
         for i, name in enumerate(out_names)}
        for c in range(n_cores)
    ]


def time_kernel(repeats=8):
    """Per-execution device time via queued-dispatch slope (ns)."""
    import jax
    import time
    sharded, concat_in, concat_zeros = LAST_RESULT["runner"]

    def run_n(n):
        t0 = time.perf_counter()
        outs = [sharded(*concat_in, *concat_zeros) for _ in range(n)]
        jax.block_until_ready(outs)
        return time.perf_counter() - t0

    run_n(1)  # warm
    t1 = min(run_n(1) for _ in range(3))
    tn = min(run_n(1 + repeats) for _ in range(2))
    dt = (tn - t1) / repeats
    LAST_RESULT["t1_s"] = t1
    LAST_RESULT["tn_s"] = tn
    return dt * 1e9
